# revision 31
# baseline (speedup 1.0000x reference)
"""Trainium2 Bass kernel for EquivariantAttention (sparse_attention).

Full (unsharded) inputs in, full output out. Internally shards over the 8
NeuronCores as (batch, T-half): core c handles batch b = c // 2, query rows
t0 = (c % 2) * 256 .. t0+256.  Every core runs the identical SPMD program on
its own input slices; there is no cross-core communication (LN and out_proj
are row-local in (b, t)).

Device-side per core:
  scores_T[s,t] = bias_T + (k_T.T @ q_T)          (bias preloaded into PSUM
                                                   via identity matmul, QK
                                                   accumulates on top)
  m = (scores_T + 20) * law_T                     (one DVE pass, fp32)
  e = exp(m - 20)  -> bf16                        (ACT, free affine bias)
  g = e * law_T                                   (DVE/gpsimd, bf16)
  numer[j,t] = sum_s v_ext[s,j] * g[s,t]          (bf16 matmul, rows 0..95)
  denom[t]   = sum_s e[s,t]                       (ones-matmul)
  per head-group, overlapped with the next group's attention:
    drain numer, recip denom (f16), broadcast via f16 matmul, divide,
    remap heads -> channel-major, square (ACT) + ssq f16 matmul accum
  tail: inorm = rsqrt(SSQ/512 + 1e-3), icol via PE transpose,
        out = (attn @ Wln.T) * inorm

Scheduling: head-group-major DMA layouts so hg0's working set (~1MB)
arrives first; resident data for hg k+1 streams during hg k's compute.
f32r spacer matmuls at kernel start hold the PE HAM clock-gate warm.
"""

import numpy as np
import ml_dtypes

import concourse.bass as bass
import concourse.bacc as bacc
import concourse.tile as tile
from concourse import mybir
from concourse.bass_utils import run_bass_kernel_spmd

# Problem constants (hardcoded per contract)
B, T, P, HID = 4, 512, 3, 512
H, D = 16, 32
EXP = 256
S = T + EXP            # 768
SCALING = (D / 3.0) ** 0.5 / D
SMOOTH = 20.0
EPS = 1e-3

NCORES = 8
TQ = T // 2            # 256 query rows per core
DH = P * D             # 96 head dim
NST = S // 128         # 6 s-tiles of 128
HG = 4                 # head groups of 4 heads

F32 = mybir.dt.float32
F16 = mybir.dt.float16
F32R = mybir.dt.float32r
BF16 = mybir.dt.bfloat16
I32 = mybir.dt.int32
AF = mybir.ActivationFunctionType
ALU = mybir.AluOpType

_CACHED_NC = None


def build_nc():
    nc = bacc.Bacc("TRN2", target_bir_lowering=False, debug=False)

    # ---- DRAM I/O (per-core shapes) ----
    # bias rows are (hg, st, s%128); cols are (4 heads, 256 t) -> one
    # 2KB-per-partition DMA per (hg, st) tile
    d_bias = nc.dram_tensor("biasT", [HG * NST * 128, 4 * TQ], F16,
                            kind="ExternalInput").ap()
    # law deduplicated: [s%128, (st, t)]
    d_law = nc.dram_tensor("lawT", [128, NST * TQ], F32,
                           kind="ExternalInput").ap()
    d_qT = nc.dram_tensor("qT", [DH, H * TQ], BF16, kind="ExternalInput").ap()
    # kTe rows are (hg, d); cols are (st, 4 heads, 128 s') -> per-hg
    # contiguous 6KB-per-partition blocks
    d_kTe = nc.dram_tensor("kTe", [HG * DH, NST * 4 * 128], BF16,
                           kind="ExternalInput").ap()
    d_vb = nc.dram_tensor("vb", [T, P * HID], BF16, kind="ExternalInput").ap()
    # same V rows, columns 4*DH..: separate tensor because indirect DMA
    # requires a zero source offset
    d_vbh = nc.dram_tensor("vbh", [T, P * HID - 4 * DH], BF16,
                           kind="ExternalInput").ap()
    d_vidx = nc.dram_tensor("vidx", [2, 128, 1], I32, kind="ExternalInput").ap()
    d_wT = nc.dram_tensor("wT", [HID, HID], F16, kind="ExternalInput").ap()
    d_id = nc.dram_tensor("ident", [128, 128], F16, kind="ExternalInput").ap()
    d_out = nc.dram_tensor("out", [TQ, P, HID], F32, kind="ExternalOutput").ap()

    with tile.TileContext(nc) as tc:
        build_kernel(tc, d_bias, d_law, d_qT, d_kTe, d_vb, d_vbh, d_vidx,
                     d_wT, d_id, d_out)
    nc.compile()
    return nc


def emit_ssq_mm(nc, post, psum1, sqs, ssq_sb, ones16, hg):
    """ssq ones-matmul over a head group's square tiles + SBUF fold.

    DVE ops cannot take two PSUM sources, so bounce through SBUF first.
    """
    ssq_ps = psum1.tile([1, 512], F32, space="PSUM", tag="ssqt",
                        name=f"ssqp{hg}")
    for i in range(2):
        nc.tensor.matmul(out=ssq_ps[0:1, :],
                         lhsT=ones16[0:96, :], rhs=sqs[i][:],
                         start=(i == 0), stop=(i == 1))
    row = post.tile([1, 2 * TQ], F32, tag="ssqrow")
    nc.vector.tensor_copy(row[:], ssq_ps[0:1, :])
    if hg == 0:
        nc.vector.tensor_tensor(out=ssq_sb[:], in0=row[0:1, 0:TQ],
                                in1=row[0:1, TQ:2 * TQ], op=ALU.add)
    else:
        tmp = post.tile([1, TQ], F32, tag="ssqtmp")
        nc.vector.tensor_tensor(out=tmp[:], in0=row[0:1, 0:TQ],
                                in1=row[0:1, TQ:2 * TQ], op=ALU.add)
        nc.vector.tensor_tensor(out=ssq_sb[:], in0=ssq_sb[:],
                                in1=tmp[:], op=ALU.add)


def emit_dn(nc, numer, denb, v_sb, ones_b, prev, hg):
    """Denominator + numerator matmuls for iteration `prev` (pipelined)."""
    st, eg2 = prev
    for half in range(2):
        e, g = eg2[half]
        nc.tensor.matmul(
            out=denb[half][0:1, :],
            lhsT=ones_b[:], rhs=e[:, :],
            start=(st == 0), stop=(st == NST - 1))
        for i2 in range(2):
            h = hg * 4 + half * 2 + i2
            nc.tensor.matmul(
                out=numer[half][:96, i2 * TQ:(i2 + 1) * TQ],
                lhsT=v_sb[st][:, h * DH:h * DH + 96],
                rhs=g[:, i2 * TQ:(i2 + 1) * TQ],
                start=(st == 0 and i2 == 0),
                stop=(st == NST - 1 and i2 == 1))


def build_kernel(tc, d_bias, d_law, d_qT, d_kTe, d_vb, d_vbh, d_vidx,
                 d_wT, d_id, d_out):
    nc = tc.nc
    from contextlib import ExitStack
    ctx = ExitStack()
    with ctx:
        const = ctx.enter_context(tc.tile_pool(name="const", bufs=1))
        big = ctx.enter_context(tc.tile_pool(name="big", bufs=1))
        biasp = ctx.enter_context(tc.tile_pool(name="biasp", bufs=8))
        work = ctx.enter_context(tc.tile_pool(name="work", bufs=2))
        post = ctx.enter_context(tc.tile_pool(name="post", bufs=2))
        attnp = ctx.enter_context(tc.tile_pool(name="attnp", bufs=1))
        psum = ctx.enter_context(tc.tile_pool(name="psum", bufs=3, space="PSUM"))
        psum1 = ctx.enter_context(tc.tile_pool(name="psum1", bufs=1, space="PSUM"))

        # ---- constants (no DMA needed, first) ----
        ones_b = const.tile([128, 1], BF16, tag="ones_b")
        nc.vector.memset(ones_b[:], 1.0)
        ones_f = const.tile([128, 1], F32, tag="ones_f")
        nc.vector.memset(ones_f[:], 1.0)
        ones16 = const.tile([128, 1], F16, tag="ones16")
        nc.vector.memset(ones16[:], 1.0)
        neg20 = const.tile([128, 1], F32, tag="neg20")
        nc.vector.memset(neg20[:], -SMOOTH)
        ones96 = const.tile([1, 96], F16, tag="ones96")
        nc.vector.memset(ones96[:], 1.0)
        # spacer scratch (memset, no DMA dependency)
        sc_f = const.tile([128, 512], F32, tag="scf")
        nc.vector.memset(sc_f[:], 0.0)
        sc_b = const.tile([128, 256], BF16, tag="scb")
        nc.gpsimd.memset(sc_b[:], 0.0)

        ident = const.tile([128, 128], F16, tag="ident")
        law = const.tile([128, NST * 2 * TQ], F32, tag="law")  # (s%128,(st,i,t))
        lawb = const.tile([128, NST * 2 * TQ], BF16, tag="lawb")
        qT = const.tile([DH, H * TQ], BF16, tag="qT")
        kte = [big.tile([DH, NST * 4 * 128], BF16, tag=f"kte{g}",
                        name=f"kte{g}") for g in range(HG)]
        wT = const.tile([128, 4 * HID], F16, tag="wT")          # (c%128,(ci,o))
        idx_sb = const.tile([128, 2], I32, tag="idx")
        v_sb = [const.tile([128, P * HID], BF16, tag=f"v{st}", name=f"v{st}")
                for st in range(6)]

        # ---- HAM warm-up: f32r spacers run ~1.7us each cold, holding the
        # PE clock-gate open until real matmuls start ----
        spacers = []
        for i in range(6):
            sp = psum.tile([128, 512], F32, space="PSUM", tag="scores",
                           name=f"sp{i}")
            nc.tensor.matmul(out=sp[0:1, :], lhsT=ones_f[:].bitcast(F32R),
                             rhs=sc_f[:].bitcast(F32R), start=True, stop=True)
            spacers.append(sp)
        for i in range(6):
            sp = psum.tile([128, 512], F32, space="PSUM", tag="scores",
                           name=f"spb{i}")
            nc.tensor.matmul(out=sp[0:1, 0:256], lhsT=ones_b[:],
                             rhs=sc_b[:], start=True, stop=True)

        # ---- startup-critical loads, spread across 4 DMA queues ----
        bias_pre = {}
        with tc.high_priority():
            # sync queue: bias hg0 (st0/st1 split in halves so the first
            # preload matmul gates on 128KB only)
            for st0 in (0, 1):
                bt = biasp.tile([128, 4 * TQ], F16, tag="bias",
                                name=f"biaspre{st0}")
                for hf in range(2):
                    nc.sync.dma_start(
                        out=bt[:, hf * 512:(hf + 1) * 512],
                        in_=d_bias[st0 * 128:(st0 + 1) * 128,
                                   hf * 512:(hf + 1) * 512])
                bias_pre[st0] = bt
            # scalar queue: ident, then hg0 residents in need order
            nc.scalar.dma_start(out=ident[:], in_=d_id)
            nc.scalar.dma_start(out=kte[0][:, 0:1024],
                                in_=d_kTe[0:DH, 0:1024])
            for c in range(2):
                nc.scalar.dma_start(out=qT[:, c * 512:(c + 1) * 512],
                                    in_=d_qT[:, c * 512:(c + 1) * 512])
            # gpsimd queue: vidx, law st0-2
            nc.gpsimd.dma_start(
                out=idx_sb[:].rearrange("p (two one) -> p two one", one=1),
                in_=d_vidx.rearrange("two p one -> p two one"))
            for c in range(3):
                nc.gpsimd.dma_start(out=law[:, c * 512:c * 512 + 256],
                                    in_=d_law[:, c * 256:(c + 1) * 256])

        # paced spacers: fire as the first real tiles arrive, bridging the
        # gap between the immediate spacers and the first real matmul
        sp = psum.tile([128, 512], F32, space="PSUM", tag="scores",
                       name="spk")
        nc.tensor.matmul(out=sp[0:1, 0:512], lhsT=ones_b[0:DH, :],
                         rhs=kte[0][:, 0:512], start=True, stop=True)
        sp = psum.tile([128, 512], F32, space="PSUM", tag="scores",
                       name="spq")
        nc.tensor.matmul(out=sp[0:1, 0:512], lhsT=ones_b[0:DH, :],
                         rhs=qT[:, 0:512], start=True, stop=True)

        # hg0 residents interleaved by first-use time on scalar queue
        nc.scalar.dma_start(out=v_sb[0][:, 0:4 * DH],
                            in_=d_vb[0:128, 0:4 * DH])
        nc.scalar.dma_start(out=kte[0][:, 1024:2048],
                            in_=d_kTe[0:DH, 1024:2048])
        nc.scalar.dma_start(out=v_sb[1][:, 0:4 * DH],
                            in_=d_vb[128:256, 0:4 * DH])
        nc.scalar.dma_start(out=v_sb[2][:, 0:4 * DH],
                            in_=d_vb[256:384, 0:4 * DH])
        nc.scalar.dma_start(out=kte[0][:, 2048:3072],
                            in_=d_kTe[0:DH, 2048:3072])
        nc.scalar.dma_start(out=v_sb[3][:, 0:4 * DH],
                            in_=d_vb[384:512, 0:4 * DH])
        # remaining bias hg0 tiles, with law st3-5 interleaved on sync
        for st0 in (2, 3, 4, 5):
            bt = biasp.tile([128, 4 * TQ], F16, tag="bias",
                            name=f"biaspre{st0}")
            nc.sync.dma_start(out=bt[:],
                              in_=d_bias[st0 * 128:(st0 + 1) * 128, :])
            bias_pre[st0] = bt
            if st0 <= 4:
                c = st0 + 1
                nc.sync.dma_start(out=law[:, c * 512:c * 512 + 256],
                                  in_=d_law[:, c * 256:(c + 1) * 256])
        # gathered v tiles (PBC expansion) from DRAM rows, on gpsimd
        for gi in range(2):
            nc.gpsimd.indirect_dma_start(
                out=v_sb[4 + gi][:], out_offset=None,
                in_=d_vb[:, :],
                in_offset=bass.IndirectOffsetOnAxis(
                    ap=idx_sb[:, gi:gi + 1], axis=0))
        # law duplication (i-slot) on ACT + bf16 cast on DVE
        for st in range(NST):
            nc.scalar.copy(law[:, st * 512 + 256:(st + 1) * 512],
                           law[:, st * 512:st * 512 + 256])
            nc.vector.tensor_copy(lawb[:, st * 512:(st + 1) * 512],
                                  law[:, st * 512:(st + 1) * 512])
        # next-hg resident data: kTe hg1 + qT hg1
        nc.scalar.dma_start(out=kte[1][:], in_=d_kTe[DH:2 * DH, :])
        nc.scalar.dma_start(out=qT[:, 4 * TQ:8 * TQ],
                            in_=d_qT[:, 4 * TQ:8 * TQ])

        # ssq accumulator in SBUF; per-hg PSUM partials drain into it
        ssq_sb = const.tile([1, TQ], F32, tag="ssq_sb")

        # channel-major divided attention, f16: [128(c%128), (p,t)]
        attn_ct = []
        for ci in range(4):
            attn_ct.append(attnp.tile([128, P * TQ], F16, tag=f"act{ci}",
                                      name=f"act{ci}"))

        # ================= attention main loop =================
        sq_tiles = [[] for _ in range(HG)]
        for hg in range(HG):
            # per-hg psum accumulators (live across st loop)
            numer = [psum1.tile([128, 2 * TQ], F32, space="PSUM",
                                tag=f"numer_{i}",
                                name=f"numer{hg}_{i}") for i in range(2)]
            denb = [psum1.tile([1, 2 * TQ], F32, space="PSUM",
                               tag=f"denb{j}", name=f"denb{hg}_{j}")
                    for j in range(2)]
            prev = None
            for st in range(NST):
                if hg == 0:
                    bt = bias_pre[st]
                else:
                    bt = biasp.tile([128, 4 * TQ], F16, tag="bias")
                    nc.sync.dma_start(
                        out=bt[:],
                        in_=d_bias[(hg * NST + st) * 128:
                                   (hg * NST + st + 1) * 128, :])
                # just-in-time prefetch of later head-groups' resident data
                if st == 1 and hg < 2:
                    nc.scalar.dma_start(out=kte[hg + 2][:],
                                        in_=d_kTe[(hg + 2) * DH:
                                                  (hg + 3) * DH, :])
                if st == 2 and hg < 2:
                    nc.scalar.dma_start(
                        out=qT[:, (hg + 2) * 4 * TQ:(hg + 3) * 4 * TQ],
                        in_=d_qT[:, (hg + 2) * 4 * TQ:(hg + 3) * 4 * TQ])
                if st < 4 and hg < 3:
                    c0 = (hg + 1) * 4 * DH
                    nc.sync.dma_start(
                        out=v_sb[st][:, c0:c0 + 4 * DH],
                        in_=d_vb[st * 128:(st + 1) * 128, c0:c0 + 4 * DH])
                if st == 4 and hg == 1:
                    nc.scalar.dma_start(
                        out=wT[:].rearrange("p (ci o) -> p ci o", ci=4),
                        in_=d_wT.rearrange("(ci p) o -> p ci o", p=128))

                law_st = law[:, st * 2 * TQ:(st + 1) * 2 * TQ]
                lawb_st = lawb[:, st * 2 * TQ:(st + 1) * 2 * TQ]
                # --- PE: bias preload + QK for this st (both halves) ---
                sc2 = []
                for half in range(2):
                    scores = psum.tile([128, 2 * TQ], F32, space="PSUM",
                                       tag="scores")
                    nc.tensor.matmul(
                        out=scores[:, :],
                        lhsT=ident[:],
                        rhs=bt[:, half * 512:(half + 1) * 512],
                        start=True, stop=False)
                    for i2 in range(2):
                        hl = half * 2 + i2
                        nc.tensor.matmul(
                            out=scores[:, i2 * TQ:(i2 + 1) * TQ],
                            lhsT=kte[hg][:, (st * 4 + hl) * 128:
                                         (st * 4 + hl + 1) * 128],
                            rhs=qT[:, (hg * 4 + hl) * TQ:
                                     (hg * 4 + hl + 1) * TQ],
                            start=False, stop=(i2 == 1))
                    sc2.append(scores)
                # --- PE: den/numer for the PREVIOUS st (inputs ready) ---
                if prev is not None:
                    emit_dn(nc, numer, denb, v_sb, ones_b, prev, hg)
                # --- elementwise for this st ---
                eg2 = []
                for half in range(2):
                    scores = sc2[half]
                    m = work.tile([128, 2 * TQ], F16, tag="m", bufs=6)
                    nc.vector.scalar_tensor_tensor(
                        out=m[:], in0=scores[:, :], scalar=SMOOTH,
                        in1=law_st, op0=ALU.add, op1=ALU.mult)
                    e = work.tile([128, 2 * TQ], BF16, tag="e", bufs=6)
                    nc.scalar.activation(e[:], m[:], AF.Exp, bias=neg20[:],
                                         scale=1.0)
                    g = work.tile([128, 2 * TQ], BF16, tag="g", bufs=6)
                    geng = nc.vector if half == 1 else nc.gpsimd
                    geng.tensor_tensor(out=g[:], in0=e[:, :], in1=lawb_st,
                                       op=ALU.mult)
                    eg2.append((e, g))
                prev = (st, eg2)
            if hg == 3:
                # pre-trigger the Sqrt ACT-table load so the tail's inorm
                # sqrt doesn't pay it on the critical path
                dum = post.tile([1, 1], F32, tag="dumsq")
                nc.scalar.activation(dum[:], ones_f[0:1, 0:1], AF.Sqrt,
                                     bias=0.0, scale=1.0)
            emit_dn(nc, numer, denb, v_sb, ones_b, prev, hg)

            # ---- per-hg postprocessing (overlaps next hg's st loop) ----
            for i in range(2):
                # drain numerator fast (frees the bank), recip denominators
                araw = post.tile([96, 2 * TQ], F32, tag=f"araw{i}")
                nc.scalar.copy(araw[:], numer[i][:96, :])
                rec2 = post.tile([1, 2 * TQ], F32, tag=f"rec2{i}")
                nc.vector.reciprocal_approx_fast(
                    out=rec2[:, :], in_=denb[i][0:1, :])
                rec16 = post.tile([1, 2 * TQ], F16, tag=f"rec16{i}")
                nc.vector.tensor_copy(rec16[:], rec2[:, :])
                # broadcast recip row to 96 partitions via f16 ones outer
                # product, reusing the den bank freed by the recip read
                rcpb = psum1.tile([96, 2 * TQ], F32, space="PSUM",
                                  tag=f"denb{i}", name=f"rcpb{hg}_{i}")
                nc.tensor.matmul(out=rcpb[:, :], lhsT=ones96[:],
                                 rhs=rec16[:, :], start=True, stop=True)
                # divide -> f16 (split per head so remap starts earlier).
                # numer partitions are (dd, p) so the head remap to
                # channel-major (dd on partitions, p free) is ONE
                # linear-order-matching DMA per head.
                ad = post.tile([96, 2 * TQ], F16, tag=f"adiv{i}")
                for j in range(2):
                    nc.vector.tensor_tensor(
                        out=ad[:, j * TQ:(j + 1) * TQ],
                        in0=araw[:, j * TQ:(j + 1) * TQ],
                        in1=rcpb[:, j * TQ:(j + 1) * TQ], op=ALU.mult)
                    hl = i * 2 + j
                    eng = (nc.sync, nc.scalar, nc.gpsimd, nc.scalar)[hl]
                    eng.dma_start(
                        out=attn_ct[hg][hl * 32:(hl + 1) * 32, :],
                        in_=ad[:, j * TQ:(j + 1) * TQ])
                # squares for ssq straight from ad (pre-remap), on DVE
                # right after the divides (inputs hot, no FIFO blocking)
                sq = post.tile([96, 2 * TQ], F16, tag=f"sq{i}")
                nc.vector.tensor_tensor(out=sq[:], in0=ad[:], in1=ad[:],
                                        op=ALU.mult)
                sq_tiles[hg].append(sq)
            # ssq matmuls DEFERRED by one head group: hg-1's squares are
            # long since ready, so these never stall the PE FIFO at the
            # head-group boundary (hg3's run in the tail)
            if hg > 0:
                emit_ssq_mm(nc, post, psum1, sq_tiles[hg - 1], ssq_sb,
                            ones16, hg - 1)

        # ================= tail =================
        # hg3's deferred ssq matmuls first: their LN scalar chain then
        # overlaps the out_proj matmuls
        emit_ssq_mm(nc, post, psum1, sq_tiles[3], ssq_sb, ones16, 3)
        # LN scalar chain (DVE/ACT) while out_proj matmuls run
        arow = const.tile([1, TQ], F32, tag="arow")
        nc.vector.tensor_scalar(
            out=arow[:], in0=ssq_sb[0:1, :], scalar1=1.0 / HID, scalar2=EPS,
            op0=ALU.mult, op1=ALU.add)
        rcpa = const.tile([1, TQ], F32, tag="rcpa")
        rscr = const.tile([1, TQ], F32, tag="rscr")
        nc.vector.reciprocal_approx_accurate(out=rcpa[:], in_=arow[:],
                                             scratch=rscr[:])
        inorm = const.tile([1, TQ], F32, tag="inorm")
        nc.scalar.activation(inorm[:], rcpa[:], AF.Sqrt, bias=0.0, scale=1.0)

        ops = []
        optags = ["scores", "scores", "scores", "numer_0", "numer_1", "denb0"]
        for k in range(6):          # tp-tiles: p = k//2, t-half = k%2
            pool_k = psum if k < 3 else psum1
            op = pool_k.tile([128, HID], F32, space="PSUM", tag=optags[k],
                             name=f"op{k}")
            for ci in range(4):
                nc.tensor.matmul(
                    out=op[:, :],
                    lhsT=attn_ct[ci][:, k * 128:(k + 1) * 128],
                    rhs=wT[:, ci * HID:(ci + 1) * HID],
                    start=(ci == 0), stop=(ci == 3))
            ops.append(op)
        # inorm row -> column via PE transpose (no DRAM bounce)
        icol_ps = psum1.tile([128, 512], F32, space="PSUM", tag="denb1",
                             name="icolps")
        for th in range(2):
            nc.tensor.transpose(out=icol_ps[:, th:th + 1],
                                in_=inorm[0:1, th * 128:(th + 1) * 128],
                                identity=ones_f[0:1, 0:1])
        icol = const.tile([128, 2], F32, tag="icol")
        nc.vector.tensor_copy(icol[:], icol_ps[:, 0:2])

        # drain out_proj with the inorm fold
        for k in range(6):          # tp-tiles: p = k//2, t-half = k%2
            op = ops[k]
            ot = work.tile([128, HID], F32, tag="osb", bufs=6)
            if k % 2 == 0:
                nc.vector.tensor_scalar(
                    out=ot[:], in0=op[:, :],
                    scalar1=icol[:, k % 2:k % 2 + 1],
                    scalar2=None, op0=ALU.mult)
            else:
                nc.scalar.mul(ot[:], op[:, :], icol[:, k % 2:k % 2 + 1])
            engs = (nc.sync, nc.scalar, nc.gpsimd)
            for oh in range(2):
                engs[(2 * k + oh) % 3].dma_start(
                    out=d_out[(k % 2) * 128 + oh * 64:
                              (k % 2) * 128 + (oh + 1) * 64, k // 2, :],
                    in_=ot[oh * 64:(oh + 1) * 64, :])


def _host_prep(q, k, v, attn_bias, local_attention_weight, out_proj_w,
               ln_weight, outcell_index):
    """Pure layout marshalling on host -> per-core input dicts."""
    q = np.asarray(q, np.float32)
    k = np.asarray(k, np.float32)
    v = np.asarray(v, np.float32)
    attn_bias = np.asarray(attn_bias, np.float32)
    law = np.asarray(local_attention_weight, np.float32)
    out_proj_w = np.asarray(out_proj_w, np.float32)
    ln_weight = np.asarray(ln_weight, np.float32)
    idx = np.asarray(outcell_index).astype(np.int64)

    # (B,T,P,HID) -> (B, 96, H, T) with row j = p*32+dd
    def to_dT(x):
        return np.ascontiguousarray(
            x.reshape(B, T, P, H, D).transpose(0, 2, 4, 3, 1)
        ).reshape(B, P * D, H, T)

    qT = to_dT(q) * np.float32(SCALING)
    kT = to_dT(k)
    # K PBC expansion along token axis (gather columns)
    kTe = np.concatenate(
        [kT, np.take_along_axis(
            kT, idx[:, None, None, :].astype(np.int64), axis=3)], axis=3)
    biasT = np.ascontiguousarray(
        attn_bias.transpose(0, 3, 1, 2)).astype(np.float16)       # (B,S,H,T)
    lawT = np.ascontiguousarray(law.transpose(0, 2, 1))            # (B,S,T)
    # head-major V columns: (B, T, (h, dd, p)) -- (dd, p) within each head
    # so numer partitions come out (dd, p), making the channel-major remap
    # a single linear DMA per head
    vb = np.ascontiguousarray(
        v.reshape(B, T, P, H, D).transpose(0, 1, 3, 4, 2)
    ).reshape(B, T, P * HID).astype(ml_dtypes.bfloat16)
    wT = np.ascontiguousarray(out_proj_w.T) * ln_weight[:, None]   # (c,o)
    wT = np.ascontiguousarray(wT, np.float32).astype(np.float16)
    vidx = idx.astype(np.int32).reshape(B, 2, 128, 1)

    in_maps = []
    for c in range(NCORES):
        b, th = c // 2, c % 2
        t0 = th * TQ
        # bias: [S=768, H=16, TQ] -> [(hg, st, 128), (4h, t)]
        bc = biasT[b, :, :, t0:t0 + TQ]                  # [768, 16, 256]
        bc = bc.reshape(NST, 128, HG, 4, TQ).transpose(2, 0, 1, 3, 4)
        bc = np.ascontiguousarray(bc).reshape(HG * NST * 128, 4 * TQ)
        # law: [S, TQ] -> [128, (st, t)] deduplicated
        lawc = lawT[b, :, t0:t0 + TQ].reshape(NST, 128, TQ).transpose(1, 0, 2)
        lawc = np.ascontiguousarray(lawc).reshape(128, NST * TQ)
        # kTe: [96, H, S] -> [(hg, 96), (st, 4h, 128)]
        kc = kTe[b].reshape(DH, HG, 4, NST, 128).transpose(1, 0, 3, 2, 4)
        kc = np.ascontiguousarray(kc).reshape(HG * DH, NST * 4 * 128)
        in_maps.append(dict(
            biasT=bc,
            lawT=lawc,
            qT=np.ascontiguousarray(
                qT[b, :, :, t0:t0 + TQ]).reshape(DH, H * TQ)
                .astype(ml_dtypes.bfloat16),
            kTe=kc.astype(ml_dtypes.bfloat16),
            vb=np.ascontiguousarray(vb[b]),
            vbh=np.ascontiguousarray(vb[b][:, 4 * DH:]),
            vidx=np.ascontiguousarray(vidx[b]),
            wT=wT,
            ident=np.eye(128, dtype=np.float16),
        ))
    return in_maps


def kernel(**inputs):
    global _CACHED_NC
    if _CACHED_NC is None:
        _CACHED_NC = build_nc()
    nc = _CACHED_NC
    in_maps = _host_prep(
        inputs["q"], inputs["k"], inputs["v"], inputs["attn_bias"],
        inputs["local_attention_weight"], inputs["out_proj_w"],
        inputs["ln_weight"], inputs["outcell_index"])
    res = run_bass_kernel_spmd(nc, in_maps, core_ids=list(range(NCORES)))
    out = np.empty((B, T, P, HID), np.float32)
    for c in range(NCORES):
        b, th = c // 2, c % 2
        out[b, th * TQ:(th + 1) * TQ] = res.results[c]["out"]
    return out


# revision 34
# speedup vs baseline: 1.1176x; 1.1176x over previous
"""Trainium2 Bass kernel for EquivariantAttention (sparse_attention).

Full (unsharded) inputs in, full output out. Internally shards over the 8
NeuronCores as (batch, T-half): core c handles batch b = c // 2, query rows
t0 = (c % 2) * 256 .. t0+256.  Every core runs the identical SPMD program on
its own input slices; there is no cross-core communication (LN and out_proj
are row-local in (b, t)).

Device-side per core:
  scores_T[s,t] = bias_T + (k_T.T @ q_T)          (bias preloaded into PSUM
                                                   via identity matmul, QK
                                                   accumulates on top)
  m = (scores_T + 20) * law_T                     (one DVE pass, fp32)
  e = exp(m - 20)  -> bf16                        (ACT, free affine bias)
  g = e * law_T                                   (DVE/gpsimd, bf16)
  numer[j,t] = sum_s v_ext[s,j] * g[s,t]          (bf16 matmul, rows 0..95)
  denom[t]   = sum_s e[s,t]                       (ones-matmul)
  per head-group, overlapped with the next group's attention:
    drain numer, recip denom (f16), broadcast via f16 matmul, divide,
    remap heads -> channel-major, square (ACT) + ssq f16 matmul accum
  tail: inorm = rsqrt(SSQ/512 + 1e-3), icol via PE transpose,
        out = (attn @ Wln.T) * inorm

Scheduling: head-group-major DMA layouts so hg0's working set (~1MB)
arrives first; resident data for hg k+1 streams during hg k's compute.
f32r spacer matmuls at kernel start hold the PE HAM clock-gate warm.
"""

import numpy as np
import ml_dtypes

import concourse.bass as bass
import concourse.bacc as bacc
import concourse.tile as tile
from concourse import mybir
from concourse.bass_utils import run_bass_kernel_spmd

# Problem constants (hardcoded per contract)
B, T, P, HID = 4, 512, 3, 512
H, D = 16, 32
EXP = 256
S = T + EXP            # 768
SCALING = (D / 3.0) ** 0.5 / D
SMOOTH = 20.0
EPS = 1e-3

NCORES = 8
TQ = T // 2            # 256 query rows per core
DH = P * D             # 96 head dim
NST = S // 128         # 6 s-tiles of 128
HG = 4                 # head groups of 4 heads

F32 = mybir.dt.float32
F16 = mybir.dt.float16
F32R = mybir.dt.float32r
BF16 = mybir.dt.bfloat16
I32 = mybir.dt.int32
AF = mybir.ActivationFunctionType
ALU = mybir.AluOpType

_CACHED_NC = None


def build_nc():
    nc = bacc.Bacc("TRN2", target_bir_lowering=False, debug=False)

    # ---- DRAM I/O (per-core shapes) ----
    # bias rows are (hg, st, s%128); cols are (4 heads, 256 t) -> one
    # 2KB-per-partition DMA per (hg, st) tile
    d_bias = nc.dram_tensor("biasT", [HG * NST * 128, 4 * TQ], F16,
                            kind="ExternalInput").ap()
    # law deduplicated: [s%128, (st, t)]
    d_law = nc.dram_tensor("lawT", [128, NST * TQ], F32,
                           kind="ExternalInput").ap()
    d_qT = nc.dram_tensor("qT", [DH, H * TQ], BF16, kind="ExternalInput").ap()
    # kTe rows are (hg, d); cols are (st, 4 heads, 128 s') -> per-hg
    # contiguous 6KB-per-partition blocks
    d_kTe = nc.dram_tensor("kTe", [HG * DH, NST * 4 * 128], BF16,
                           kind="ExternalInput").ap()
    d_vb = nc.dram_tensor("vb", [T, P * HID], BF16, kind="ExternalInput").ap()
    # same V rows, columns 4*DH..: separate tensor because indirect DMA
    # requires a zero source offset
    d_vbh = nc.dram_tensor("vbh", [T, P * HID - 4 * DH], BF16,
                           kind="ExternalInput").ap()
    d_vidx = nc.dram_tensor("vidx", [2, 128, 1], I32, kind="ExternalInput").ap()
    d_wT = nc.dram_tensor("wT", [HID, HID], F16, kind="ExternalInput").ap()
    d_id = nc.dram_tensor("ident", [128, 128], F16, kind="ExternalInput").ap()
    d_out = nc.dram_tensor("out", [TQ, P, HID], F32, kind="ExternalOutput").ap()

    with tile.TileContext(nc) as tc:
        build_kernel(tc, d_bias, d_law, d_qT, d_kTe, d_vb, d_vbh, d_vidx,
                     d_wT, d_id, d_out)
    nc.compile()
    return nc


def emit_ssq_mm(nc, post, psum1, sqs, ssq_sb, ones16, hg):
    """ssq ones-matmul over a head group's square tiles + SBUF fold.

    DVE ops cannot take two PSUM sources, so bounce through SBUF first.
    """
    ssq_ps = psum1.tile([1, 512], F32, space="PSUM", tag="ssqt",
                        name=f"ssqp{hg}")
    for i in range(2):
        nc.tensor.matmul(out=ssq_ps[0:1, :],
                         lhsT=ones16[0:96, :], rhs=sqs[i][:],
                         start=(i == 0), stop=(i == 1))
    row = post.tile([1, 2 * TQ], F32, tag="ssqrow")
    nc.vector.tensor_copy(row[:], ssq_ps[0:1, :])
    if hg == 0:
        nc.vector.tensor_tensor(out=ssq_sb[:], in0=row[0:1, 0:TQ],
                                in1=row[0:1, TQ:2 * TQ], op=ALU.add)
    else:
        tmp = post.tile([1, TQ], F32, tag="ssqtmp")
        nc.vector.tensor_tensor(out=tmp[:], in0=row[0:1, 0:TQ],
                                in1=row[0:1, TQ:2 * TQ], op=ALU.add)
        nc.vector.tensor_tensor(out=ssq_sb[:], in0=ssq_sb[:],
                                in1=tmp[:], op=ALU.add)


def emit_dn(nc, numer, denb, v_sb, ones_b, prev, hg):
    """Denominator + numerator matmuls for iteration `prev` (pipelined)."""
    st, eg2 = prev
    for half in range(2):
        e, g = eg2[half]
        nc.tensor.matmul(
            out=denb[half][0:1, :],
            lhsT=ones_b[:], rhs=e[:, :],
            start=(st == 0), stop=(st == NST - 1))
        for i2 in range(2):
            h = hg * 4 + half * 2 + i2
            nc.tensor.matmul(
                out=numer[half][:96, i2 * TQ:(i2 + 1) * TQ],
                lhsT=v_sb[st][:, h * DH:h * DH + 96],
                rhs=g[:, i2 * TQ:(i2 + 1) * TQ],
                start=(st == 0 and i2 == 0),
                stop=(st == NST - 1 and i2 == 1))


def build_kernel(tc, d_bias, d_law, d_qT, d_kTe, d_vb, d_vbh, d_vidx,
                 d_wT, d_id, d_out):
    nc = tc.nc
    from contextlib import ExitStack
    ctx = ExitStack()
    with ctx:
        const = ctx.enter_context(tc.tile_pool(name="const", bufs=1))
        big = ctx.enter_context(tc.tile_pool(name="big", bufs=1))
        biasp = ctx.enter_context(tc.tile_pool(name="biasp", bufs=8))
        work = ctx.enter_context(tc.tile_pool(name="work", bufs=2))
        post = ctx.enter_context(tc.tile_pool(name="post", bufs=2))
        attnp = ctx.enter_context(tc.tile_pool(name="attnp", bufs=1))
        psum = ctx.enter_context(tc.tile_pool(name="psum", bufs=3, space="PSUM"))
        psum1 = ctx.enter_context(tc.tile_pool(name="psum1", bufs=1, space="PSUM"))

        # ---- constants (no DMA needed, first) ----
        ones_b = const.tile([128, 1], BF16, tag="ones_b")
        nc.vector.memset(ones_b[:], 1.0)
        ones_f = const.tile([128, 1], F32, tag="ones_f")
        nc.vector.memset(ones_f[:], 1.0)
        ones16 = const.tile([128, 1], F16, tag="ones16")
        nc.vector.memset(ones16[:], 1.0)
        neg20 = const.tile([128, 1], F32, tag="neg20")
        nc.vector.memset(neg20[:], -SMOOTH)
        ones96 = const.tile([1, 96], F16, tag="ones96")
        nc.vector.memset(ones96[:], 1.0)
        # spacer scratch (memset, no DMA dependency)
        sc_f = const.tile([128, 512], F32, tag="scf")
        nc.vector.memset(sc_f[:], 0.0)
        sc_b = const.tile([128, 256], BF16, tag="scb")
        nc.gpsimd.memset(sc_b[:], 0.0)

        ident = const.tile([128, 128], F16, tag="ident")
        law = const.tile([128, NST * 2 * TQ], F32, tag="law")  # (s%128,(st,i,t))
        lawb = const.tile([128, NST * 2 * TQ], BF16, tag="lawb")
        qT = const.tile([DH, H * TQ], BF16, tag="qT")
        kte = [big.tile([DH, NST * 4 * 128], BF16, tag=f"kte{g}",
                        name=f"kte{g}") for g in range(HG)]
        wT = const.tile([128, 4 * HID], F16, tag="wT")          # (c%128,(ci,o))
        idx_sb = const.tile([128, 2], I32, tag="idx")
        v_sb = [const.tile([128, P * HID], BF16, tag=f"v{st}", name=f"v{st}")
                for st in range(6)]

        # ---- HAM warm-up: f32r spacers run ~1.7us each cold, holding the
        # PE clock-gate open until real matmuls start ----
        spacers = []
        for i in range(6):
            sp = psum.tile([128, 512], F32, space="PSUM", tag="scores",
                           name=f"sp{i}")
            nc.tensor.matmul(out=sp[0:1, :], lhsT=ones_f[:].bitcast(F32R),
                             rhs=sc_f[:].bitcast(F32R), start=True, stop=True)
            spacers.append(sp)
        for i in range(6):
            sp = psum.tile([128, 512], F32, space="PSUM", tag="scores",
                           name=f"spb{i}")
            nc.tensor.matmul(out=sp[0:1, 0:256], lhsT=ones_b[:],
                             rhs=sc_b[:], start=True, stop=True)

        # ---- startup-critical loads, spread across 4 DMA queues ----
        bias_pre = {}
        with tc.high_priority():
            # sync queue: bias hg0 (st0/st1 split in halves so the first
            # preload matmul gates on 128KB only)
            for st0 in (0, 1):
                bt = biasp.tile([128, 4 * TQ], F16, tag="bias",
                                name=f"biaspre{st0}")
                for hf in range(2):
                    nc.sync.dma_start(
                        out=bt[:, hf * 512:(hf + 1) * 512],
                        in_=d_bias[st0 * 128:(st0 + 1) * 128,
                                   hf * 512:(hf + 1) * 512])
                bias_pre[st0] = bt
            # scalar queue: ident, then hg0 residents in need order
            nc.scalar.dma_start(out=ident[:], in_=d_id)
            nc.scalar.dma_start(out=kte[0][:, 0:1024],
                                in_=d_kTe[0:DH, 0:1024])
            for c in range(2):
                nc.scalar.dma_start(out=qT[:, c * 512:(c + 1) * 512],
                                    in_=d_qT[:, c * 512:(c + 1) * 512])
            # gpsimd queue: vidx, law st0-2
            nc.gpsimd.dma_start(
                out=idx_sb[:].rearrange("p (two one) -> p two one", one=1),
                in_=d_vidx.rearrange("two p one -> p two one"))
            for c in range(3):
                nc.gpsimd.dma_start(out=law[:, c * 512:c * 512 + 256],
                                    in_=d_law[:, c * 256:(c + 1) * 256])

        # paced spacers: fire as the first real tiles arrive, bridging the
        # gap between the immediate spacers and the first real matmul
        sp = psum.tile([128, 512], F32, space="PSUM", tag="scores",
                       name="spk")
        nc.tensor.matmul(out=sp[0:1, 0:512], lhsT=ones_b[0:DH, :],
                         rhs=kte[0][:, 0:512], start=True, stop=True)
        sp = psum.tile([128, 512], F32, space="PSUM", tag="scores",
                       name="spq")
        nc.tensor.matmul(out=sp[0:1, 0:512], lhsT=ones_b[0:DH, :],
                         rhs=qT[:, 0:512], start=True, stop=True)

        # hg0 residents interleaved by first-use time on scalar queue
        nc.scalar.dma_start(out=v_sb[0][:, 0:4 * DH],
                            in_=d_vb[0:128, 0:4 * DH])
        nc.scalar.dma_start(out=kte[0][:, 1024:2048],
                            in_=d_kTe[0:DH, 1024:2048])
        nc.scalar.dma_start(out=v_sb[1][:, 0:4 * DH],
                            in_=d_vb[128:256, 0:4 * DH])
        nc.scalar.dma_start(out=v_sb[2][:, 0:4 * DH],
                            in_=d_vb[256:384, 0:4 * DH])
        nc.scalar.dma_start(out=kte[0][:, 2048:3072],
                            in_=d_kTe[0:DH, 2048:3072])
        nc.scalar.dma_start(out=v_sb[3][:, 0:4 * DH],
                            in_=d_vb[384:512, 0:4 * DH])
        # remaining bias hg0 tiles, with law st3-5 interleaved on sync
        for st0 in (2, 3, 4, 5):
            bt = biasp.tile([128, 4 * TQ], F16, tag="bias",
                            name=f"biaspre{st0}")
            nc.sync.dma_start(out=bt[:],
                              in_=d_bias[st0 * 128:(st0 + 1) * 128, :])
            bias_pre[st0] = bt
            if st0 <= 4:
                c = st0 + 1
                nc.sync.dma_start(out=law[:, c * 512:c * 512 + 256],
                                  in_=d_law[:, c * 256:(c + 1) * 256])
        # gathered v tiles (PBC expansion) from DRAM rows, on gpsimd
        for gi in range(2):
            nc.gpsimd.indirect_dma_start(
                out=v_sb[4 + gi][:], out_offset=None,
                in_=d_vb[:, :],
                in_offset=bass.IndirectOffsetOnAxis(
                    ap=idx_sb[:, gi:gi + 1], axis=0))
        # law duplication (i-slot) on ACT + bf16 cast on DVE
        for st in range(NST):
            nc.scalar.copy(law[:, st * 512 + 256:(st + 1) * 512],
                           law[:, st * 512:st * 512 + 256])
            nc.vector.tensor_copy(lawb[:, st * 512:(st + 1) * 512],
                                  law[:, st * 512:(st + 1) * 512])
        # next-hg resident data: kTe hg1 + qT hg1
        nc.scalar.dma_start(out=kte[1][:], in_=d_kTe[DH:2 * DH, :])
        nc.scalar.dma_start(out=qT[:, 4 * TQ:8 * TQ],
                            in_=d_qT[:, 4 * TQ:8 * TQ])

        # ssq accumulator in SBUF; per-hg PSUM partials drain into it
        ssq_sb = const.tile([1, TQ], F32, tag="ssq_sb")

        # channel-major divided attention, f16: [128(c%128), (p,t)]
        attn_ct = []
        for ci in range(4):
            attn_ct.append(attnp.tile([128, P * TQ], F16, tag=f"act{ci}",
                                      name=f"act{ci}"))

        # ================= attention main loop =================
        # Post-processing of head group k is software-pipelined into head
        # group k+1's st-loop (st0: numer drain + recip; st1: rcpb matmuls
        # + divides + remap; st2: squares + ssq reduce) so the PE FIFO
        # never blocks on the cross-engine recip chain at hg boundaries.
        def post_part0(ph):
            hgp, numerp, denbp = ph
            parts = {}
            for i in range(2):
                araw = post.tile([96, 2 * TQ], F32, tag=f"araw{i}")
                nc.scalar.copy(araw[:], numerp[i][:96, :])
                rec2 = post.tile([1, 2 * TQ], F32, tag=f"rec2{i}")
                nc.vector.reciprocal_approx_fast(
                    out=rec2[:, :], in_=denbp[i][0:1, :])
                rec16 = post.tile([1, 2 * TQ], F16, tag=f"rec16{i}")
                nc.vector.tensor_copy(rec16[:], rec2[:, :])
                parts[i] = (araw, rec16)
            return parts

        def post_part1(ph, parts):
            hgp = ph[0]
            ads = {}
            for i in range(2):
                araw, rec16 = parts[i]
                # broadcast recip row to 96 partitions via f16 ones outer
                # product, reusing the den bank freed by the recip read
                rcpb = psum1.tile([96, 2 * TQ], F32, space="PSUM",
                                  tag=f"denb{i}", name=f"rcpb{hgp}_{i}")
                nc.tensor.matmul(out=rcpb[:, :], lhsT=ones96[:],
                                 rhs=rec16[:, :], start=True, stop=True)
                # divide -> f16. numer partitions are (dd, p) so the head
                # remap to channel-major is ONE linear DMA per head.
                ad = post.tile([96, 2 * TQ], F16, tag=f"adiv{i}")
                for j in range(2):
                    nc.vector.tensor_tensor(
                        out=ad[:, j * TQ:(j + 1) * TQ],
                        in0=araw[:, j * TQ:(j + 1) * TQ],
                        in1=rcpb[:, j * TQ:(j + 1) * TQ], op=ALU.mult)
                    hl = i * 2 + j
                    eng = (nc.sync, nc.scalar, nc.gpsimd, nc.scalar)[hl]
                    eng.dma_start(
                        out=attn_ct[hgp][hl * 32:(hl + 1) * 32, :],
                        in_=ad[:, j * TQ:(j + 1) * TQ])
                ads[i] = ad
            return ads

        def post_part2(ph, ads, sq_eng):
            hgp = ph[0]
            sqs = []
            for i in range(2):
                sq = post.tile([96, 2 * TQ], F16, tag=f"sq{i}")
                sq_eng.tensor_tensor(out=sq[:], in0=ads[i][:],
                                     in1=ads[i][:], op=ALU.mult)
                sqs.append(sq)
            emit_ssq_mm(nc, post, psum1, sqs, ssq_sb, ones16, hgp)

        post_q = None
        pp0 = pp1 = None
        for hg in range(HG):
            # per-hg psum accumulators (live across st loop)
            numer = [psum1.tile([128, 2 * TQ], F32, space="PSUM",
                                tag=f"numer_{i}",
                                name=f"numer{hg}_{i}") for i in range(2)]
            denb = [psum1.tile([1, 2 * TQ], F32, space="PSUM",
                               tag=f"denb{j}", name=f"denb{hg}_{j}")
                    for j in range(2)]
            prev = None
            for st in range(NST):
                if hg == 0:
                    bt = bias_pre[st]
                else:
                    bt = biasp.tile([128, 4 * TQ], F16, tag="bias")
                    nc.sync.dma_start(
                        out=bt[:],
                        in_=d_bias[(hg * NST + st) * 128:
                                   (hg * NST + st + 1) * 128, :])
                # just-in-time prefetch of later head-groups' resident data
                if st == 1 and hg < 2:
                    nc.scalar.dma_start(out=kte[hg + 2][:],
                                        in_=d_kTe[(hg + 2) * DH:
                                                  (hg + 3) * DH, :])
                if st == 2 and hg < 2:
                    nc.scalar.dma_start(
                        out=qT[:, (hg + 2) * 4 * TQ:(hg + 3) * 4 * TQ],
                        in_=d_qT[:, (hg + 2) * 4 * TQ:(hg + 3) * 4 * TQ])
                if st < 4 and hg < 3:
                    c0 = (hg + 1) * 4 * DH
                    nc.sync.dma_start(
                        out=v_sb[st][:, c0:c0 + 4 * DH],
                        in_=d_vb[st * 128:(st + 1) * 128, c0:c0 + 4 * DH])
                if st == 4 and hg == 1:
                    nc.scalar.dma_start(
                        out=wT[:].rearrange("p (ci o) -> p ci o", ci=4),
                        in_=d_wT.rearrange("(ci p) o -> p ci o", p=128))

                law_st = law[:, st * 2 * TQ:(st + 1) * 2 * TQ]
                lawb_st = lawb[:, st * 2 * TQ:(st + 1) * 2 * TQ]
                # --- PE: bias preload + QK for this st (both halves) ---
                sc2 = []
                for half in range(2):
                    scores = psum.tile([128, 2 * TQ], F32, space="PSUM",
                                       tag="scores")
                    nc.tensor.matmul(
                        out=scores[:, :],
                        lhsT=ident[:],
                        rhs=bt[:, half * 512:(half + 1) * 512],
                        start=True, stop=False)
                    for i2 in range(2):
                        hl = half * 2 + i2
                        nc.tensor.matmul(
                            out=scores[:, i2 * TQ:(i2 + 1) * TQ],
                            lhsT=kte[hg][:, (st * 4 + hl) * 128:
                                         (st * 4 + hl + 1) * 128],
                            rhs=qT[:, (hg * 4 + hl) * TQ:
                                     (hg * 4 + hl + 1) * TQ],
                            start=False, stop=(i2 == 1))
                    sc2.append(scores)
                # --- PE: den/numer for the PREVIOUS st (inputs ready) ---
                if prev is not None:
                    emit_dn(nc, numer, denb, v_sb, ones_b, prev, hg)
                # --- pipelined post-processing of the previous hg ---
                if post_q is not None:
                    if st == 0:
                        pp0 = post_part0(post_q)
                    elif st == 1:
                        pp1 = post_part1(post_q, pp0)
                    elif st == 2:
                        post_part2(post_q, pp1, nc.gpsimd)
                        post_q = None
                # --- elementwise for this st ---
                eg2 = []
                for half in range(2):
                    scores = sc2[half]
                    m = work.tile([128, 2 * TQ], F16, tag="m", bufs=6)
                    nc.vector.scalar_tensor_tensor(
                        out=m[:], in0=scores[:, :], scalar=SMOOTH,
                        in1=law_st, op0=ALU.add, op1=ALU.mult)
                    e = work.tile([128, 2 * TQ], BF16, tag="e", bufs=6)
                    nc.scalar.activation(e[:], m[:], AF.Exp, bias=neg20[:],
                                         scale=1.0)
                    g = work.tile([128, 2 * TQ], BF16, tag="g", bufs=6)
                    geng = nc.vector if half == 1 else nc.gpsimd
                    geng.tensor_tensor(out=g[:], in0=e[:, :], in1=lawb_st,
                                       op=ALU.mult)
                    eg2.append((e, g))
                prev = (st, eg2)
            if hg == 3:
                # pre-trigger the Sqrt ACT-table load so the tail's inorm
                # sqrt doesn't pay it on the critical path
                dum = post.tile([1, 1], F32, tag="dumsq")
                nc.scalar.activation(dum[:], ones_f[0:1, 0:1], AF.Sqrt,
                                     bias=0.0, scale=1.0)
            emit_dn(nc, numer, denb, v_sb, ones_b, prev, hg)
            post_q = (hg, numer, denb)

        # ================= tail =================
        # hg3's post chain interleaved with out_proj: channel blocks 0-2
        # (ready long ago) hide the drain/recip/divide latency on the PE.
        pp0 = post_part0(post_q)
        ops = []
        optags = ["scores", "scores", "scores", "numer_0", "numer_1"]
        for k in range(5):          # tp-tiles: p = k//2, t-half = k%2
            pool_k = psum if k < 3 else psum1
            op = pool_k.tile([128, HID], F32, space="PSUM", tag=optags[k],
                             name=f"op{k}")
            for ci in range(3):
                nc.tensor.matmul(
                    out=op[:, :],
                    lhsT=attn_ct[ci][:, k * 128:(k + 1) * 128],
                    rhs=wT[:, ci * HID:(ci + 1) * HID],
                    start=(ci == 0), stop=False)
            ops.append(op)
        pp1 = post_part1(post_q, pp0)
        # k=5 op reuses the denb0 bank; its alloc waits the i=0 divides
        op = psum1.tile([128, HID], F32, space="PSUM", tag="denb0",
                        name="op5")
        for ci in range(3):
            nc.tensor.matmul(
                out=op[:, :],
                lhsT=attn_ct[ci][:, 5 * 128:6 * 128],
                rhs=wT[:, ci * HID:(ci + 1) * HID],
                start=(ci == 0), stop=False)
        ops.append(op)
        # squares + ssq + LN scalar chain (overlaps the matmuls above)
        post_part2(post_q, pp1, nc.vector)
        arow = const.tile([1, TQ], F32, tag="arow")
        nc.vector.tensor_scalar(
            out=arow[:], in0=ssq_sb[0:1, :], scalar1=1.0 / HID, scalar2=EPS,
            op0=ALU.mult, op1=ALU.add)
        rcpa = const.tile([1, TQ], F32, tag="rcpa")
        rscr = const.tile([1, TQ], F32, tag="rscr")
        nc.vector.reciprocal_approx_accurate(out=rcpa[:], in_=arow[:],
                                             scratch=rscr[:])
        inorm = const.tile([1, TQ], F32, tag="inorm")
        nc.scalar.activation(inorm[:], rcpa[:], AF.Sqrt, bias=0.0, scale=1.0)
        # final ci=3 accumulation once the hg3 head remap has landed
        for k in range(6):
            nc.tensor.matmul(
                out=ops[k][:, :],
                lhsT=attn_ct[3][:, k * 128:(k + 1) * 128],
                rhs=wT[:, 3 * HID:4 * HID],
                start=False, stop=True)
        # inorm row -> column via PE transpose (no DRAM bounce)
        icol_ps = psum1.tile([128, 512], F32, space="PSUM", tag="denb1",
                             name="icolps")
        for th in range(2):
            nc.tensor.transpose(out=icol_ps[:, th:th + 1],
                                in_=inorm[0:1, th * 128:(th + 1) * 128],
                                identity=ones_f[0:1, 0:1])
        icol = const.tile([128, 2], F32, tag="icol")
        nc.vector.tensor_copy(icol[:], icol_ps[:, 0:2])

        # drain out_proj with the inorm fold
        for k in range(6):          # tp-tiles: p = k//2, t-half = k%2
            op = ops[k]
            ot = work.tile([128, HID], F32, tag="osb", bufs=6)
            if k % 2 == 0:
                nc.vector.tensor_scalar(
                    out=ot[:], in0=op[:, :],
                    scalar1=icol[:, k % 2:k % 2 + 1],
                    scalar2=None, op0=ALU.mult)
            else:
                nc.scalar.mul(ot[:], op[:, :], icol[:, k % 2:k % 2 + 1])
            engs = (nc.sync, nc.scalar, nc.gpsimd)
            for oh in range(2):
                engs[(2 * k + oh) % 3].dma_start(
                    out=d_out[(k % 2) * 128 + oh * 64:
                              (k % 2) * 128 + (oh + 1) * 64, k // 2, :],
                    in_=ot[oh * 64:(oh + 1) * 64, :])


def _host_prep(q, k, v, attn_bias, local_attention_weight, out_proj_w,
               ln_weight, outcell_index):
    """Pure layout marshalling on host -> per-core input dicts."""
    q = np.asarray(q, np.float32)
    k = np.asarray(k, np.float32)
    v = np.asarray(v, np.float32)
    attn_bias = np.asarray(attn_bias, np.float32)
    law = np.asarray(local_attention_weight, np.float32)
    out_proj_w = np.asarray(out_proj_w, np.float32)
    ln_weight = np.asarray(ln_weight, np.float32)
    idx = np.asarray(outcell_index).astype(np.int64)

    # (B,T,P,HID) -> (B, 96, H, T) with row j = p*32+dd
    def to_dT(x):
        return np.ascontiguousarray(
            x.reshape(B, T, P, H, D).transpose(0, 2, 4, 3, 1)
        ).reshape(B, P * D, H, T)

    qT = to_dT(q) * np.float32(SCALING)
    kT = to_dT(k)
    # K PBC expansion along token axis (gather columns)
    kTe = np.concatenate(
        [kT, np.take_along_axis(
            kT, idx[:, None, None, :].astype(np.int64), axis=3)], axis=3)
    biasT = np.ascontiguousarray(
        attn_bias.transpose(0, 3, 1, 2)).astype(np.float16)       # (B,S,H,T)
    lawT = np.ascontiguousarray(law.transpose(0, 2, 1))            # (B,S,T)
    # head-major V columns: (B, T, (h, dd, p)) -- (dd, p) within each head
    # so numer partitions come out (dd, p), making the channel-major remap
    # a single linear DMA per head
    vb = np.ascontiguousarray(
        v.reshape(B, T, P, H, D).transpose(0, 1, 3, 4, 2)
    ).reshape(B, T, P * HID).astype(ml_dtypes.bfloat16)
    wT = np.ascontiguousarray(out_proj_w.T) * ln_weight[:, None]   # (c,o)
    wT = np.ascontiguousarray(wT, np.float32).astype(np.float16)
    vidx = idx.astype(np.int32).reshape(B, 2, 128, 1)

    in_maps = []
    for c in range(NCORES):
        b, th = c // 2, c % 2
        t0 = th * TQ
        # bias: [S=768, H=16, TQ] -> [(hg, st, 128), (4h, t)]
        bc = biasT[b, :, :, t0:t0 + TQ]                  # [768, 16, 256]
        bc = bc.reshape(NST, 128, HG, 4, TQ).transpose(2, 0, 1, 3, 4)
        bc = np.ascontiguousarray(bc).reshape(HG * NST * 128, 4 * TQ)
        # law: [S, TQ] -> [128, (st, t)] deduplicated
        lawc = lawT[b, :, t0:t0 + TQ].reshape(NST, 128, TQ).transpose(1, 0, 2)
        lawc = np.ascontiguousarray(lawc).reshape(128, NST * TQ)
        # kTe: [96, H, S] -> [(hg, 96), (st, 4h, 128)]
        kc = kTe[b].reshape(DH, HG, 4, NST, 128).transpose(1, 0, 3, 2, 4)
        kc = np.ascontiguousarray(kc).reshape(HG * DH, NST * 4 * 128)
        in_maps.append(dict(
            biasT=bc,
            lawT=lawc,
            qT=np.ascontiguousarray(
                qT[b, :, :, t0:t0 + TQ]).reshape(DH, H * TQ)
                .astype(ml_dtypes.bfloat16),
            kTe=kc.astype(ml_dtypes.bfloat16),
            vb=np.ascontiguousarray(vb[b]),
            vbh=np.ascontiguousarray(vb[b][:, 4 * DH:]),
            vidx=np.ascontiguousarray(vidx[b]),
            wT=wT,
            ident=np.eye(128, dtype=np.float16),
        ))
    return in_maps


def kernel(**inputs):
    global _CACHED_NC
    if _CACHED_NC is None:
        _CACHED_NC = build_nc()
    nc = _CACHED_NC
    in_maps = _host_prep(
        inputs["q"], inputs["k"], inputs["v"], inputs["attn_bias"],
        inputs["local_attention_weight"], inputs["out_proj_w"],
        inputs["ln_weight"], inputs["outcell_index"])
    res = run_bass_kernel_spmd(nc, in_maps, core_ids=list(range(NCORES)))
    out = np.empty((B, T, P, HID), np.float32)
    for c in range(NCORES):
        b, th = c // 2, c % 2
        out[b, th * TQ:(th + 1) * TQ] = res.results[c]["out"]
    return out
